# revision 10
# baseline (speedup 1.0000x reference)
"""Trainium2 Bass kernel for NodeFeatureExtractor (v2).

Key idea: bilinear interpolation is linear, so interp(map) @ W1a ==
interp(map @ W1a).  The host folds the 484-channel feature map through
W1's big block once (one 16384x484 @ 484x128 matmul), leaving a
128-channel pre-folded map.  The device then only:
  - indirect-DMA gathers one 1KB block per node (2x2 bilinear footprint,
    stored as [f00, f01-f00, f10, f11-f10] x 128ch bf16)
  - separable lerp on DVE (5 big strided/broadcast ops per chunk)
  - PE: transpose-accumulate of the lerped features onto the structural
    matmul ([cx,cy,deg,dist] @ W1b^T) in PSUM, relu, W2 matmul, relu
  - writes h2 in [hid, node] layout (host transposes back)
Degree histogram (exact, collision-free) and the degree max stay on the
host as in the baseline (HW scatter-add loses colliding RMWs); with the
counts host-side the global max is host-side too, so no collective.

Data-parallel over nodes: each of the 8 cores runs the same program on
its 25088-node shard; the folded map + weights are replicated.
"""
import threading
from contextlib import ExitStack

import numpy as np
import ml_dtypes

import bass_rust
import concourse.bass as bass
import concourse.bacc as bacc
import concourse.mybir as mybir
import concourse.tile as tile
from concourse import masks

F32 = mybir.dt.float32
BF16 = mybir.dt.bfloat16
I16 = mybir.dt.int16
ALU = mybir.AluOpType
ACTF = mybir.ActivationFunctionType

BF16_NP = ml_dtypes.bfloat16

N_CORES = 8
HID = 128
FH = FW = 128
NPIX = FH * FW          # 16384
BLK = 4 * HID           # 512 values per gathered node block
NCH = 512               # nodes per MLP sub-chunk (one PSUM bank wide)
GCH = 3584              # max nodes per gather chunk (one SBUF tile)
GSUB = 896              # nodes per dma_gather instruction (HW-safe <=1024)


class CFG:
    def __init__(self, n_shard, n_cores, image_size=512.0):
        assert n_shard % NCH == 0
        self.n_shard = n_shard
        self.n_cores = n_cores
        self.pad_n = n_shard * n_cores
        self.image_size = float(image_size)
        self.chunks = []
        off = 0
        while off < n_shard:
            c = min(GCH, n_shard - off)
            self.chunks.append((off, c))
            off += c


NQ = 4                  # SWDGE queues (parallelize Q7 descriptor-gen)


def build_nc(cfg: CFG) -> bass.Bass:
    nc = bacc.Bacc("TRN2", num_devices=cfg.n_cores, num_swdge_queues=NQ)
    ns = cfg.n_shard
    npc = ns // 128

    map2 = nc.dram_tensor("map2", [NPIX, BLK], BF16, kind="ExternalInput")
    idx_in = nc.dram_tensor("idx_in", [128, ns // 16], I16, kind="ExternalInput")
    wx_in = nc.dram_tensor("wx_in", [128, npc], F32, kind="ExternalInput")
    wy_in = nc.dram_tensor("wy_in", [128, npc], F32, kind="ExternalInput")
    s_in = nc.dram_tensor("s_in", [4, ns], BF16, kind="ExternalInput")
    w1b_in = nc.dram_tensor("w1b_in", [4, 128], BF16, kind="ExternalInput")
    w2T_in = nc.dram_tensor("w2T_in", [128, 128], BF16, kind="ExternalInput")
    b1_in = nc.dram_tensor("b1_in", [128, 1], F32, kind="ExternalInput")
    b2_in = nc.dram_tensor("b2_in", [128, 1], F32, kind="ExternalInput")
    h_outT = nc.dram_tensor("h_outT", [128, ns], BF16, kind="ExternalOutput")

    gsrc = bass_rust.AP(map2[:, :].tensor, 0, [[BLK, NPIX], [1, BLK]])

    with tile.TileContext(nc) as tc, ExitStack() as ctx:
        st = ctx.enter_context(tc.tile_pool(name="static", bufs=1))
        gpool = ctx.enter_context(tc.tile_pool(name="gather", bufs=2))
        fpool = ctx.enter_context(tc.tile_pool(name="feat", bufs=2))
        hpool = ctx.enter_context(tc.tile_pool(name="hid", bufs=2))
        ps1p = ctx.enter_context(tc.tile_pool(name="ps_1", bufs=2, space="PSUM"))
        ps2p = ctx.enter_context(tc.tile_pool(name="ps_2", bufs=2, space="PSUM"))

        # ---- static loads
        ident = st.tile([128, 128], F32)
        masks.make_identity(nc, ident[:])
        identb = st.tile([128, 128], BF16)
        nc.vector.tensor_copy(identb[:], ident[:])
        idx = st.tile([128, ns // 16], I16)
        nc.sync.dma_start(idx[:], idx_in[:, :])
        wx = st.tile([128, npc], F32)
        nc.sync.dma_start(wx[:], wx_in[:, :])
        wy = st.tile([128, npc], F32)
        nc.sync.dma_start(wy[:], wy_in[:, :])
        s_sb = st.tile([4, ns], BF16)
        nc.sync.dma_start(s_sb[:], s_in[:, :])
        w1b = st.tile([4, 128], BF16)
        nc.sync.dma_start(w1b[:], w1b_in[:, :])
        w2T = st.tile([128, 128], BF16)
        nc.sync.dma_start(w2T[:], w2T_in[:, :])
        b1 = st.tile([128, 1], F32)
        nc.sync.dma_start(b1[:], b1_in[:, :])
        b2 = st.tile([128, 1], F32)
        nc.sync.dma_start(b2[:], b2_in[:, :])

        for off, csz in cfg.chunks:
            cc = csz // 128          # node cols in this chunk
            j0 = off // 128
            # one gathered 2x2 block per node: [f00, f01-f00, f10, f11-f10]
            # (split into <=GSUB-idx gathers: large num_idxs wedges the HW)
            g = gpool.tile([128, cc, BLK], BF16, tag="g")
            for si, s0 in enumerate(range(0, csz, GSUB)):
                ssz = min(GSUB, csz - s0)
                nc.gpsimd.dma_gather(
                    g[:, (s0 // 128):(s0 + ssz) // 128, :], gsrc,
                    idx[:, (off + s0) // 16:(off + s0 + ssz) // 16],
                    ssz, ssz, BLK, queue_num=si % NQ)

            # separable bilinear lerp (in-place x-lerp into the diff slots)
            g4 = g[:, :, :].rearrange("p c (r h) -> p c r h", r=2)
            wxb = wx[:, j0:j0 + cc].unsqueeze(2).unsqueeze(3) \
                .to_broadcast([128, cc, 2, 128])
            nc.vector.tensor_tensor(g4[:, :, :, 128:256], g4[:, :, :, 128:256],
                                    wxb, ALU.mult)
            nc.vector.tensor_tensor(g4[:, :, :, 128:256], g4[:, :, :, 128:256],
                                    g4[:, :, :, 0:128], ALU.add)
            dy = fpool.tile([128, cc, 128], BF16, tag="dy")
            nc.vector.tensor_tensor(dy[:], g[:, :, 384:512], g[:, :, 128:256],
                                    ALU.subtract)
            wyb = wy[:, j0:j0 + cc].unsqueeze(2).to_broadcast([128, cc, 128])
            nc.vector.tensor_tensor(dy[:], dy[:], wyb, ALU.mult)
            feat = fpool.tile([128, cc, 128], BF16, tag="feat")
            nc.vector.tensor_tensor(feat[:], dy[:], g[:, :, 128:256], ALU.add)

            for k in range(csz // NCH):
                n0 = off + k * NCH
                ps1 = ps1p.tile([128, NCH], F32)
                nc.tensor.matmul(ps1[:], w1b[:, :], s_sb[:, n0:n0 + NCH],
                                 start=True, stop=False)
                for gi in range(4):
                    nc.tensor.matmul(ps1[:, 128 * gi:128 * (gi + 1)],
                                     feat[:, 4 * k + gi, :], identb[:, :],
                                     start=False, stop=(gi == 3))
                h1 = hpool.tile([128, NCH], BF16, tag="h1")
                nc.scalar.activation(h1[:], ps1[:], ACTF.Relu, bias=b1[:, :])
                ps2 = ps2p.tile([128, NCH], F32)
                nc.tensor.matmul(ps2[:], w2T[:, :], h1[:], start=True,
                                 stop=True)
                h2 = hpool.tile([128, NCH], BF16, tag="h2")
                nc.scalar.activation(h2[:], ps2[:], ACTF.Relu, bias=b2[:, :])
                nc.sync.dma_start(h_outT[:, n0:n0 + NCH], h2[:])

    nc.compile()
    return nc


# ---------------- host side ----------------

def prep_inputs(cfg: CFG, vertices, backbone_features, seg_probs, edge_index,
                W1, b1, W2, b2):
    """Host prep: W1 fold, block map, indices/weights, degree, layouts."""
    im = cfg.image_size
    v = np.asarray(vertices, np.float32)
    n = v.shape[0]
    if n < cfg.pad_n:
        v = np.concatenate([v, np.repeat(v[-1:], cfg.pad_n - n, 0)], 0)

    W1 = np.asarray(W1, np.float32)
    # fold the backbone+seg block of W1 into the feature map
    m = np.empty((NPIX, 484), np.float32)
    m[:, :480] = np.asarray(backbone_features, np.float32).reshape(480, -1).T
    m[:, 480:] = np.asarray(seg_probs, np.float32).reshape(4, -1).T
    M1 = (m @ W1[:, 2:486].T).reshape(FH, FW, HID)          # (y, x, 128)
    x1 = np.minimum(np.arange(FW) + 1, FW - 1)
    f00 = M1
    f01 = M1[:, x1]
    f10 = M1[np.minimum(np.arange(FH) + 1, FH - 1)]
    f11 = f10[:, x1]
    map2 = np.stack([f00, f01 - f00, f10, f11 - f10], axis=2)
    map2 = np.ascontiguousarray(map2.reshape(NPIX, BLK)).astype(BF16_NP)

    # per-node bilinear cell + weights (reference's align_corners grid)
    sx = (FW - 1) / im
    ix = v[:, 0].astype(np.float64) * sx
    iy = v[:, 1].astype(np.float64) * sx
    x0 = np.clip(np.floor(ix), 0, FW - 2)
    y0 = np.clip(np.floor(iy), 0, FH - 2)
    wxv = (ix - x0).astype(np.float32)
    wyv = (iy - y0).astype(np.float32)
    pix = (y0 * FW + x0).astype(np.int16)

    # exact degree histogram + global max (device scatter-add loses
    # colliding RMWs, so the histogram lives host-side as in the baseline)
    ep = np.asarray(edge_index).reshape(-1).astype(np.int64)
    degree = np.bincount(ep, minlength=cfg.pad_n).astype(np.float32)
    deg_n = degree / (degree[:n].max() + 1e-6)
    dx = np.minimum(v[:, 0], im - v[:, 0])
    dyb = np.minimum(v[:, 1], im - v[:, 1])
    dist = np.minimum(dx, dyb) / (im / 2)
    S = np.stack([v[:, 0] / im, v[:, 1] / im, deg_n, dist]).astype(BF16_NP)

    w1b = np.ascontiguousarray(W1[:, [0, 1, 486, 487]].T).astype(BF16_NP)
    w2T = np.ascontiguousarray(np.asarray(W2, np.float32).T).astype(BF16_NP)
    b1c = np.ascontiguousarray(np.asarray(b1, np.float32).reshape(128, 1))
    b2c = np.ascontiguousarray(np.asarray(b2, np.float32).reshape(128, 1))

    in_maps = []
    ns = cfg.n_shard
    for c in range(cfg.n_cores):
        sl = slice(c * ns, (c + 1) * ns)
        in_maps.append({
            "map2": map2,
            "idx_in": np.ascontiguousarray(
                np.tile(pix[sl].reshape(-1, 16).T, (8, 1))),
            "wx_in": np.ascontiguousarray(wxv[sl].reshape(-1, 128).T),
            "wy_in": np.ascontiguousarray(wyv[sl].reshape(-1, 128).T),
            "s_in": np.ascontiguousarray(S[:, sl]),
            "w1b_in": w1b, "w2T_in": w2T, "b1_in": b1c, "b2_in": b2c,
        })
    return in_maps


_NC_CACHE: dict = {}
_NC_LOCK = threading.Lock()


def kernel(vertices, backbone_features, seg_probs, edge_index, W1, b1, W2, b2,
           image_size):
    from concourse.bass_utils import run_bass_kernel_spmd

    n = int(np.asarray(vertices).shape[0])
    n_shard = -(-n // (N_CORES * NCH)) * NCH
    cfg = CFG(n_shard, N_CORES, float(np.asarray(image_size)))

    key = (cfg.n_shard, cfg.n_cores, cfg.image_size)
    with _NC_LOCK:
        if key not in _NC_CACHE:
            _NC_CACHE[key] = build_nc(cfg)
        nc = _NC_CACHE[key]

    in_maps = prep_inputs(cfg, vertices, backbone_features, seg_probs,
                          edge_index, W1, b1, W2, b2)
    res = run_bass_kernel_spmd(nc, in_maps, core_ids=list(range(N_CORES)))
    h = np.concatenate(
        [res.results[c]["h_outT"].T for c in range(N_CORES)], 0)
    return np.ascontiguousarray(h[:n]).astype(np.float32)
